# revision 1
# baseline (speedup 1.0000x reference)
"""Trainium2 Bass kernel for nn_EngramShortConv (RMSNorm + depthwise dilated
causal conv1d + silu), 8-core SPMD.

  x: [B=4, L=4096, HC=4, D=1024] fp32 -> y same shape/dtype.

Sharding: 16 independent (b, hc) groups, 2 per NeuronCore, zero communication.

Per core, per 512-token chunk (natural [token, d] layout from HBM, fp16
on-chip, fp32 PSUM accumulation):
  1. stats: DVE scalar_tensor_tensor squares x with free-dim accumulate
     -> sum(x^2) per token; ACT sqrt + DVE reciprocal -> r = rsqrt(ms+eps).
  2. pass1 (PE): Z[d, t] = X_blk^T @ diag(r) per 128x128 block -- the
     transpose to channel-major with the RMSNorm scale folded in for free.
     DVE/ACT copy PSUM -> SBUF fp16 with a 6-column halo from the previous
     chunk (causal left pad).
  3. pass2 (PE): the depthwise conv as 4 PSUM-accumulated matmuls
     diag(conv_w[k] * norm_w) @ Z[:, t - 6 + 2k], with the norm affine
     weight folded into the diagonals on the host.
  4. ACT Silu reads conv PSUM -> fp16; pass3 (PE) transposes back via
     identity; DVE/ACT copy to SBUF; DMA out.

I/O precision: host casts x to fp16 before upload (the device would round
to fp16 anyway; halves input DMA) and the device returns fp16 y upcast to
fp32 on host. End-to-end scale-relative error ~1e-3.

Engine balance per core (~255 us measured): PE ~188 us (1536 matmuls),
DVE ~184 us (square-reduce + PSUM copies), ACT ~170 us (silu + copies +
sqrt), DMA ~119 us (34 MB at ~290 GB/s).
"""

import sys

if "/opt/trn_rl_repo" not in sys.path:
    sys.path.insert(0, "/opt/trn_rl_repo")

import numpy as np

B, L, HC, D = 4, 4096, 4, 1024
K, DIL = 4, 2
EPS = 1e-5
PAD = (K - 1) * DIL  # 6
NCORES = 8
NGROUPS = B * HC     # 16
GPC = NGROUPS // NCORES  # 2 groups per core

# tunables
TCH = 512            # tokens per chunk (= matmul moving free dim)
RSQRT_MODE = "act"   # 'pow' (DVE) or 'act' (ACT Sqrt + DVE reciprocal)
IN_F16 = True        # host casts x to f16 before upload (halves in-DMA)
OUT_F16 = True       # f16 device output, host upcasts to f32
CPAIR = 2            # chunks whose stats are batched (amortize ACT tables)
SQ_ENGINE = "vector"  # engine for the square+accumulate pass
OUTCOPY_ACT = 3      # of 4 blks per chunk, how many outcopies go to ACT
ZCOPY_ACT = 1        # of 8 dsubs per chunk, how many zcopies go to ACT

_prog_cache = {}


def build_program(L_=L, gpc=GPC, tch=TCH, rsqrt_mode=RSQRT_MODE,
                  in_f16=IN_F16, out_f16=OUT_F16, cpair=CPAIR,
                  sq_engine=SQ_ENGINE, outcopy_act=OUTCOPY_ACT,
                  zcopy_act=ZCOPY_ACT):
    """Build the per-core Bacc program. Same program on all cores (SPMD)."""
    import concourse.bacc as bacc
    import concourse.tile as tile
    from concourse import mybir

    f32 = mybir.dt.float32
    f16 = mybir.dt.float16
    AF = mybir.ActivationFunctionType
    ALU = mybir.AluOpType

    nblk = tch // 128
    dsub = D // 128
    nchunks = L_ // tch
    assert tch % 128 == 0 and L_ % tch == 0 and D % 128 == 0

    nc = bacc.Bacc()
    xin = nc.declare_dram_parameter("xin", [gpc, L_, D],
                                f16 if in_f16 else f32, isOutput=False)
    wdg = nc.declare_dram_parameter("wdg", [gpc, K, dsub, 128, 128], f16,
                                    isOutput=False)
    idn = nc.declare_dram_parameter("idn", [128, 128], f16, isOutput=False)
    yout = nc.declare_dram_parameter("yout", [gpc, L_, D],
                                 f16 if out_f16 else f32, isOutput=True)

    # views: token index t = c*tch + blk*128 + p
    xv = xin[:].rearrange("g (c blk p) d -> g c p blk d", blk=nblk, p=128)
    yv = yout[:].rearrange("g (c blk p) d -> g c p blk d", blk=nblk, p=128)
    wv = wdg[:].rearrange("g k s p m -> p g k s m")

    with tile.TileContext(nc) as tc:
        with (
            tc.tile_pool(name="pconst", bufs=1) as pconst,
            tc.tile_pool(name="px", bufs=6) as px,
            tc.tile_pool(name="pxf", bufs=2) as pxf,
            tc.tile_pool(name="pstat", bufs=3) as pstat,
            tc.tile_pool(name="pz", bufs=3) as pz,
            tc.tile_pool(name="py", bufs=3) as py,
            tc.tile_pool(name="po", bufs=2) as po,
            tc.tile_pool(name="pp1", bufs=2, space="PSUM") as pp1,
            tc.tile_pool(name="pp2", bufs=2, space="PSUM") as pp2,
            tc.tile_pool(name="pp3", bufs=2, space="PSUM") as pp3,
        ):
            ident = pconst.tile([128, 128], f16)
            nc.sync.dma_start(out=ident[:], in_=idn[:])
            wsb = pconst.tile([128, gpc, K, dsub, 128], f16)
            nc.sync.dma_start(out=wsb[:], in_=wv)
            eps_t = pconst.tile([128, 1], f32)
            nc.vector.memset(eps_t[:], EPS)

            zt_prev = None
            yo_dt = f16 if out_f16 else f32
            for g in range(gpc):
                for c0 in range(0, nchunks, cpair):
                    cs = list(range(c0, min(c0 + cpair, nchunks)))
                    ncs = len(cs)
                    # ---- load chunks (natural [token, d] layout) ----
                    xhs = []
                    for c in cs:
                        xh = px.tile([128, nblk, D], f16, tag="xh")
                        nc.sync.dma_start(out=xh[:], in_=xv[g, c])
                        xhs.append(xh)

                    # ---- stats r = (mean(x^2)+eps)^-0.5, batched ----
                    ssq = pstat.tile([128, ncs, nblk], f32, tag="ssq")
                    for j in range(ncs):
                        for blk in range(nblk):
                            scr = pstat.tile([128, D], f16, tag="scr")
                            if sq_engine == "vector":
                                nc.vector.scalar_tensor_tensor(
                                    out=scr[:], in0=xhs[j][:, blk, :],
                                    scalar=1.0, in1=xhs[j][:, blk, :],
                                    op0=ALU.mult, op1=ALU.mult,
                                    accum_out=ssq[:, j, blk:blk + 1])
                            else:
                                nc.scalar.activation(
                                    out=scr[:], in_=xhs[j][:, blk, :],
                                    func=AF.Square,
                                    accum_out=ssq[:, j, blk:blk + 1])
                    r = pstat.tile([128, ncs, nblk], f32, tag="r")
                    t1 = pstat.tile([128, ncs, nblk], f32, tag="t1")
                    nc.scalar.activation(
                        out=t1[:], in_=ssq[:], func=AF.Sqrt,
                        scale=1.0 / D, bias=eps_t[:])
                    nc.vector.reciprocal(out=r[:], in_=t1[:])

                    # ---- pass1 per chunk: Z[d, t] = X^T diag(r) ----
                    zts = []
                    for j, c in enumerate(cs):
                        xh = xhs[j]
                        drt = pstat.tile([128, nblk, 128], f16, tag="drt")
                        for blk in range(nblk):
                            nc.vector.tensor_scalar_mul(
                                out=drt[:, blk, :], in0=ident[:],
                                scalar1=r[:, j, blk:blk + 1])

                        zt = pz.tile([128, dsub, PAD + tch], f16, tag="zt")
                        if c == 0:
                            nc.vector.memset(zt[:, :, 0:PAD], 0.0)
                        else:
                            for s in range(dsub):
                                nc.vector.tensor_copy(
                                    out=zt[:, s, 0:PAD],
                                    in_=zt_prev[:, s, tch:tch + PAD])
                        for s in range(dsub):
                            zp = pp1.tile([128, tch], f32, tag="zp")
                            for blk in range(nblk):
                                nc.tensor.matmul(
                                    zp[:, blk * 128:(blk + 1) * 128],
                                    lhsT=xh[:, blk, s * 128:(s + 1) * 128],
                                    rhs=drt[:, blk, :],
                                    start=True, stop=True)
                            if s < zcopy_act:
                                nc.scalar.copy(
                                    out=zt[:, s, PAD:PAD + tch], in_=zp[:])
                            else:
                                nc.vector.tensor_copy(
                                    out=zt[:, s, PAD:PAD + tch], in_=zp[:])
                        zt_prev = zt
                        zts.append(zt)

                    # ---- pass2 paired: conv matmuls share ldweights ----
                    yhs = [py.tile([128, dsub, tch], f16, tag=f"yh{j}",
                   name=f"yh{j}")
                           for j in range(ncs)]
                    for s in range(dsub):
                        yps = [pp2.tile([128, tch], f32, tag=f"yp{j}",
                       name=f"yp{j}")
                               for j in range(ncs)]
                        for k in range(K):
                            for j in range(ncs):
                                nc.tensor.matmul(
                                    yps[j][:],
                                    lhsT=wsb[:, g, k, s, :],
                                    rhs=zts[j][:, s, k * DIL:k * DIL + tch],
                                    start=(k == 0), stop=(k == K - 1))
                        for j in range(ncs):
                            nc.scalar.activation(out=yhs[j][:, s, :],
                                                 in_=yps[j][:], func=AF.Silu)

                    # ---- pass3 per chunk: transpose back + copy + store ----
                    for j, c in enumerate(cs):
                        yh = yhs[j]
                        yo = po.tile([128, nblk, D], yo_dt, tag="yo")
                        for blk in range(nblk):
                            on_act = blk < outcopy_act
                            for half in range(2):
                                tp = pp3.tile([128, D // 2], f32, tag="tp")
                                for sh in range(dsub // 2):
                                    s = half * (dsub // 2) + sh
                                    nc.tensor.matmul(
                                        tp[:, sh * 128:(sh + 1) * 128],
                                        lhsT=yh[:, s,
                                                blk * 128:(blk + 1) * 128],
                                        rhs=ident[:],
                                        start=True, stop=True)
                                dst = yo[:, blk,
                                         half * (D // 2):(half + 1) * (D // 2)]
                                if on_act:
                                    nc.scalar.copy(out=dst, in_=tp[:])
                                else:
                                    nc.vector.tensor_copy(out=dst, in_=tp[:])
                        nc.sync.dma_start(out=yv[g, c], in_=yo[:])
    nc.compile()
    return nc


def _host_pack(x, norm_weight, conv_weight):
    """Shard inputs across cores; fold norm weight into conv diagonals."""
    dsub = D // 128
    xg = np.ascontiguousarray(x.transpose(0, 2, 1, 3)).reshape(NGROUPS, L, D)
    if IN_F16:
        xg = xg.astype(np.float16)
    conv_w = conv_weight.reshape(HC, D, K)            # [hc, d, k]
    weff = conv_w * norm_weight[:, :, None]           # [hc, d, k]
    wr = weff.transpose(0, 2, 1).reshape(HC, K, dsub, 128)  # [hc, k, s, i]
    eye = np.eye(128, dtype=np.float32)
    wdiag = (wr[..., None] * eye).astype(np.float16)  # [hc, K, s, 128, 128]
    idn = eye.astype(np.float16)

    in_maps = []
    for i in range(NCORES):
        gs = [i * GPC + j for j in range(GPC)]
        in_maps.append({
            "xin": np.ascontiguousarray(xg[gs[0]:gs[-1] + 1]),
            "wdg": np.ascontiguousarray(
                np.stack([wdiag[g % HC] for g in gs])),
            "idn": idn,
        })
    return in_maps


def _host_unpack(results):
    ys = np.concatenate([r["yout"] for r in results], axis=0)  # [16, L, D]
    y = ys.reshape(B, HC, L, D).transpose(0, 2, 1, 3)
    return np.ascontiguousarray(y.astype(np.float32))


def _get_prog():
    key = (L, GPC, TCH, RSQRT_MODE, IN_F16, OUT_F16, CPAIR, SQ_ENGINE,
           OUTCOPY_ACT, ZCOPY_ACT)
    if key not in _prog_cache:
        _prog_cache[key] = build_program()
    return _prog_cache[key]


def kernel(x, norm_weight, conv_weight, _trace=False, _trace_kwargs=None):
    from concourse.bass_utils import run_bass_kernel_spmd

    x = np.asarray(x, dtype=np.float32)
    norm_weight = np.asarray(norm_weight, dtype=np.float32)
    conv_weight = np.asarray(conv_weight, dtype=np.float32)

    nc = _get_prog()
    in_maps = _host_pack(x, norm_weight, conv_weight)
    res = run_bass_kernel_spmd(
        nc, in_maps, list(range(NCORES)),
        trace=_trace, **(_trace_kwargs or {}))
    out = _host_unpack(res.results)
    if _trace:
        return out, res
    return out



# revision 2
# speedup vs baseline: 1.1521x; 1.1521x over previous
"""Trainium2 Bass kernel for nn_EngramShortConv — v5.

v4 lessons folded in:
  - Pool compute OFF (its software SBUF traffic degraded DVE tensor_scalar
    from 4x to 2x mode)
  - 2-wide pipeline steps: both groups' unit u processed per step, so ACT
    table switches happen once per function-group per step
  - halo comes in with the input DMA (reads 6 extra tokens from HBM);
    only the tiny R tail is copied between units
  - rsqrt via one Abs_reciprocal_sqrt op (RSQRT_MODE fallback: Ln+Exp)
  - conv: PE s-blocks {3..7} (diag matmuls, PSUM), DVE {0,1,2}
    (tensor_scalar 4x + batched tensor_tensor adds); square for s7 on DVE
"""

import sys

if "/opt/trn_rl_repo" not in sys.path:
    sys.path.insert(0, "/opt/trn_rl_repo")

import numpy as np

B, L, HC, D = 4, 4096, 4, 1024
K, DIL = 4, 2
EPS = 1e-5
PAD = (K - 1) * DIL  # 6
NCORES = 8
NGROUPS = B * HC
GPC = NGROUPS // NCORES  # 2
DSUB = D // 128          # 8
UT = 1024
NU = L // UT             # 4 units/group

PE_S = (3, 4, 5, 6, 7)  # conv on PE
DVE_SQ_S = (7,)          # squares on DVE (contiguous tail)
RSQRT_MODE = "absrsqrt"  # or "lnexp"

_prog_cache = {}


def build_program(pe_s=PE_S, dve_sq_s=DVE_SQ_S, rsqrt_mode=RSQRT_MODE):
    import concourse.bacc as bacc
    import concourse.tile as tile
    from concourse import mybir

    f32 = mybir.dt.float32
    f16 = mybir.dt.float16
    AF = mybir.ActivationFunctionType
    ALU = mybir.AluOpType

    dve_s = tuple(s for s in range(DSUB) if s not in pe_s)
    ndve = len(dve_s)
    npe = len(pe_s)
    lo, hi = min(dve_s), max(dve_s) + 1
    assert dve_s == tuple(range(lo, hi))
    sq_lo = min(dve_sq_s) if dve_sq_s else DSUB
    assert dve_sq_s == tuple(range(sq_lo, DSUB))

    nc = bacc.Bacc()
    xin = nc.declare_dram_parameter("xin", [GPC, DSUB, 128, L], f16,
                                    isOutput=False)
    wts = nc.declare_dram_parameter("wts", [128, GPC, K, DSUB], f32,
                                    isOutput=False)
    ons = nc.declare_dram_parameter("ons", [128, 128], f16, isOutput=False)
    idn = nc.declare_dram_parameter("idn", [128, 128], f16, isOutput=False)
    yout = nc.declare_dram_parameter("yout", [GPC, DSUB, 128, L], f16,
                                     isOutput=True)

    xv = xin[:].rearrange("g s p t -> g p s t")
    yv = yout[:].rearrange("g s p t -> g p s t")

    with tile.TileContext(nc) as tc:
        with (
            tc.tile_pool(name="pconst", bufs=1) as pconst,
            tc.tile_pool(name="px", bufs=4) as px,
            tc.tile_pool(name="py", bufs=3) as py,
            tc.tile_pool(name="pr", bufs=6) as pr,
            tc.tile_pool(name="plt", bufs=2) as plt,
            tc.tile_pool(name="pcv", bufs=2) as pcv,
            tc.tile_pool(name="pdg", bufs=2) as pdg,
            tc.tile_pool(name="pps", bufs=2, space="PSUM") as pps,
            tc.tile_pool(name="ppc", bufs=2, space="PSUM") as ppc,
        ):
            ones = pconst.tile([128, 128], f16)
            nc.sync.dma_start(out=ones[:], in_=ons[:])
            ident = pconst.tile([128, 128], f16)
            nc.sync.dma_start(out=ident[:], in_=idn[:])
            w = pconst.tile([128, GPC, K, DSUB], f32)
            nc.sync.dma_start(out=w[:], in_=wts[:])
            eps_t = pconst.tile([128, 1], f32)
            nc.vector.memset(eps_t[:], EPS)

            xs, ys, rs, mss, yconvs, dgs = {}, {}, {}, {}, {}, {}
            rs_prev = {}

            def stage_load(g, u):
                x = px.tile([128, DSUB, PAD + UT], f16, tag="x")
                xs[(g, u)] = x
                if u == 0:
                    nc.vector.memset(x[:, :, 0:PAD], 0.0)
                    nc.sync.dma_start(
                        out=x[:, :, PAD:PAD + UT],
                        in_=xv[g, :, :, 0:UT])
                else:
                    nc.sync.dma_start(
                        out=x[:],
                        in_=xv[g, :, :, u * UT - PAD:(u + 1) * UT])

            def stage_squares_act(g, u):
                x = xs[(g, u)]
                y = py.tile([128, DSUB, UT], f16, tag="y")
                ys[(g, u)] = y
                h = sq_lo // 2
                nc.scalar.activation(out=y[:, 0:h, :],
                                     in_=x[:, 0:h, PAD:PAD + UT],
                                     func=AF.Square)
                nc.scalar.activation(out=y[:, h:sq_lo, :],
                                     in_=x[:, h:sq_lo, PAD:PAD + UT],
                                     func=AF.Square)

            def stage_squares_dve(g, u):
                if sq_lo >= DSUB:
                    return
                x, y = xs[(g, u)], ys[(g, u)]
                nc.vector.tensor_tensor(
                    out=y[:, sq_lo:DSUB, :],
                    in0=x[:, sq_lo:DSUB, PAD:PAD + UT],
                    in1=x[:, sq_lo:DSUB, PAD:PAD + UT], op=ALU.mult)

            def stage_mssum(g, u):
                y = ys[(g, u)]
                ms = pps.tile([128, UT], f32, tag="ms")
                for half in range(2):
                    hs = half * 512
                    for s in range(DSUB):
                        nc.tensor.matmul(
                            ms[:, hs:hs + 512], lhsT=ones[:],
                            rhs=y[:, s, hs:hs + 512],
                            start=(s == 0), stop=(s == DSUB - 1))
                mss[(g, u)] = ms

            def stage_rsqrt(g, u):
                ms = mss.pop((g, u))
                r = pr.tile([128, PAD + UT], f16, tag="r")
                if rsqrt_mode == "absrsqrt":
                    nc.scalar.activation(
                        out=r[:, PAD:PAD + UT], in_=ms[:],
                        func=AF.Abs_reciprocal_sqrt,
                        scale=1.0 / D, bias=eps_t[:])
                else:
                    lnt = plt.tile([128, UT], f16, tag="lnt")
                    nc.scalar.activation(out=lnt[:], in_=ms[:], func=AF.Ln,
                                         scale=1.0 / D, bias=eps_t[:])
                    nc.scalar.activation(out=r[:, PAD:PAD + UT], in_=lnt[:],
                                         func=AF.Exp, scale=-0.5)
                rs[(g, u)] = r

            def stage_halo_norm(g, u):
                x, r = xs[(g, u)], rs[(g, u)]
                if u == 0:
                    nc.vector.memset(r[:, 0:PAD], 0.0)
                else:
                    rp = rs_prev[g]
                    nc.vector.tensor_copy(out=r[:, 0:PAD],
                                          in_=rp[:, UT:UT + PAD])
                rb = r[:].rearrange("p (o t) -> p o t", o=1).broadcast_to(
                    [128, DSUB, PAD + UT])
                nc.vector.tensor_tensor(
                    out=x[:], in0=x[:], in1=rb, op=ALU.mult)
                rs_prev[g] = r

            def stage_conv_pe(g, u):
                x = xs[(g, u)]
                dg = dgs[g]
                yps = []
                for si, s in enumerate(pe_s):
                    yp = ppc.tile([128, UT], f32, tag="yp")
                    yps.append(yp)
                    for c in range(UT // 512):
                        cs = c * 512
                        for k in range(K):
                            nc.tensor.matmul(
                                yp[:, cs:cs + 512], lhsT=dg[:, si, k, :],
                                rhs=x[:, s, 2 * k + cs:2 * k + cs + 512],
                                start=(k == 0), stop=(k == K - 1))
                yconvs[(g, u)] = yps

            def stage_conv_dve(g, u):
                x, y = xs[(g, u)], ys[(g, u)]
                for si, s in enumerate(dve_s):
                    nc.vector.tensor_scalar_mul(
                        out=y[:, s, :], in0=x[:, s, 0:UT],
                        scalar1=w[:, g, 0, s:s + 1])
                for k in range(1, K):
                    tmp = pcv.tile([128, ndve, UT], f16, tag="tmp")
                    for si, s in enumerate(dve_s):
                        nc.vector.tensor_scalar_mul(
                            out=tmp[:, si, :], in0=x[:, s, 2 * k:2 * k + UT],
                            scalar1=w[:, g, k, s:s + 1])
                    nc.vector.tensor_tensor(
                        out=y[:, lo:hi, :], in0=y[:, lo:hi, :],
                        in1=tmp[:], op=ALU.add)

            def stage_finish(g, u):
                y = ys[(g, u)]
                yps = yconvs.pop((g, u))
                for si, s in enumerate(pe_s):
                    nc.scalar.activation(out=y[:, s, :], in_=yps[si][:],
                                         func=AF.Silu)
                nc.scalar.activation(out=y[:, lo:hi, :], in_=y[:, lo:hi, :],
                                     func=AF.Silu)
                nc.sync.dma_start(
                    out=yv[g, :, 0:4, u * UT:(u + 1) * UT], in_=y[:, 0:4, :])
                nc.sync.dma_start(
                    out=yv[g, :, 4:8, u * UT:(u + 1) * UT], in_=y[:, 4:8, :])

            def stage_diag(g):
                dg = pdg.tile([128, npe, K, 128], f16, tag="dg")
                dgs[g] = dg
                for si, s in enumerate(pe_s):
                    for k in range(K):
                        nc.vector.tensor_scalar_mul(
                            out=dg[:, si, k, :], in0=ident[:],
                            scalar1=w[:, g, k, s:s + 1])

            # 2-wide pipeline: both groups' unit u per step; stats of
            # u+1 interleave with compute of u, per-engine FIFOs ordered so
            # ACT table switches cluster (Square* / AbsRsqrt* / Silu*).
            GS = list(range(GPC))
            stage_diag(0)
            stage_diag(1)
            for gg in GS:
                stage_load(gg, 0)
            for gg in GS:
                stage_squares_act(gg, 0)
            for gg in GS:
                stage_squares_dve(gg, 0)
            for gg in GS:
                stage_mssum(gg, 0)
            for gg in GS:
                stage_rsqrt(gg, 0)
            for u in range(NU):
                nu = u + 1 if u + 1 < NU else None
                if nu is not None:
                    for gg in GS:
                        stage_load(gg, nu)
                    for gg in GS:
                        stage_squares_act(gg, nu)
                for gg in GS:
                    stage_halo_norm(gg, u)
                if nu is not None:
                    for gg in GS:
                        stage_squares_dve(gg, nu)
                for gg in GS:
                    stage_conv_pe(gg, u)
                if nu is not None:
                    for gg in GS:
                        stage_mssum(gg, nu)
                for gg in GS:
                    stage_conv_dve(gg, u)
                if nu is not None:
                    for gg in GS:
                        stage_rsqrt(gg, nu)
                for gg in GS:
                    stage_finish(gg, u)
    nc.compile()
    return nc


def _host_pack(x, norm_weight, conv_weight):
    xt = np.ascontiguousarray(
        x.astype(np.float16).transpose(0, 2, 3, 1)
    ).reshape(NGROUPS, DSUB, 128, L)
    weff = (conv_weight.reshape(HC, D, K)
            * norm_weight[:, :, None]).astype(np.float32)
    ones = np.ones((128, 128), dtype=np.float16)
    ident = np.eye(128, dtype=np.float16)

    in_maps = []
    for i in range(NCORES):
        gs = [i * GPC + j for j in range(GPC)]
        wc = np.stack([weff[g % HC].reshape(DSUB, 128, K) for g in gs])
        wc = np.ascontiguousarray(wc.transpose(2, 0, 3, 1))
        in_maps.append({
            "xin": np.ascontiguousarray(xt[gs[0]:gs[-1] + 1]),
            "wts": wc,
            "ons": ones,
            "idn": ident,
        })
    return in_maps


def _host_unpack(results):
    ys = np.concatenate([r["yout"] for r in results], axis=0)
    y = ys.reshape(B, HC, D, L).transpose(0, 3, 1, 2)
    return np.ascontiguousarray(y.astype(np.float32))


def _get_prog():
    key = (PE_S, DVE_SQ_S, RSQRT_MODE)
    if key not in _prog_cache:
        _prog_cache[key] = build_program()
    return _prog_cache[key]


def kernel(x, norm_weight, conv_weight, _trace=False, _trace_kwargs=None):
    from concourse.bass_utils import run_bass_kernel_spmd

    x = np.asarray(x, dtype=np.float32)
    norm_weight = np.asarray(norm_weight, dtype=np.float32)
    conv_weight = np.asarray(conv_weight, dtype=np.float32)

    nc = _get_prog()
    in_maps = _host_pack(x, norm_weight, conv_weight)
    res = run_bass_kernel_spmd(
        nc, in_maps, list(range(NCORES)),
        trace=_trace, **(_trace_kwargs or {}))
    out = _host_unpack(res.results)
    if _trace:
        return out, res
    return out


# revision 3
# speedup vs baseline: 1.2412x; 1.0773x over previous
"""Trainium2 Bass kernel for nn_EngramShortConv — v8.

v4 lessons folded in:
  - Pool compute OFF (its software SBUF traffic degraded DVE tensor_scalar
    from 4x to 2x mode)
  - 2-wide pipeline steps: both groups' unit u processed per step, so ACT
    table switches happen once per function-group per step
  - halo comes in with the input DMA (reads 6 extra tokens from HBM);
    only the tiny R tail is copied between units
  - rsqrt via one Abs_reciprocal_sqrt op (RSQRT_MODE fallback: Ln+Exp)
  - conv: PE s-blocks {3..7} (diag matmuls, PSUM), DVE {0,1,2}
    (tensor_scalar 4x + batched tensor_tensor adds); square for s7 on DVE
"""

import sys

if "/opt/trn_rl_repo" not in sys.path:
    sys.path.insert(0, "/opt/trn_rl_repo")

import numpy as np

B, L, HC, D = 4, 4096, 4, 1024
K, DIL = 4, 2
EPS = 1e-5
PAD = (K - 1) * DIL  # 6
NCORES = 8
NGROUPS = B * HC
GPC = NGROUPS // NCORES  # 2
DSUB = D // 128          # 8
UT = 1024
NU = L // UT             # 4 units/group

PE_S = (2, 3, 4, 5, 6, 7)  # conv on PE
DVE_SQ_S = (4, 5, 6, 7)  # squares on DVE (contiguous tail)
RSQRT_MODE = "absrsqrt"  # or "lnexp"

_prog_cache = {}


def build_program(pe_s=PE_S, dve_sq_s=DVE_SQ_S, rsqrt_mode=RSQRT_MODE):
    import concourse.bacc as bacc
    import concourse.tile as tile
    from concourse import mybir

    f32 = mybir.dt.float32
    f16 = mybir.dt.float16
    AF = mybir.ActivationFunctionType
    ALU = mybir.AluOpType

    dve_s = tuple(s for s in range(DSUB) if s not in pe_s)
    ndve = len(dve_s)
    npe = len(pe_s)
    lo, hi = min(dve_s), max(dve_s) + 1
    assert dve_s == tuple(range(lo, hi))
    sq_lo = min(dve_sq_s) if dve_sq_s else DSUB
    assert dve_sq_s == tuple(range(sq_lo, DSUB))

    nc = bacc.Bacc()
    xin = nc.declare_dram_parameter("xin", [GPC, DSUB, 128, L], f16,
                                    isOutput=False)
    wts = nc.declare_dram_parameter("wts", [128, GPC, K, DSUB], f32,
                                    isOutput=False)
    ons = nc.declare_dram_parameter("ons", [128, 128], f16, isOutput=False)
    idn = nc.declare_dram_parameter("idn", [128, 128], f16, isOutput=False)
    yout = nc.declare_dram_parameter("yout", [GPC, DSUB, 128, L], f16,
                                     isOutput=True)

    xv = xin[:].rearrange("g s p t -> g p s t")
    yv = yout[:].rearrange("g s p t -> g p s t")

    import concourse.bacc as _bacc_mod
    from concourse import hw_specs as _hw_specs

    _orig_gat = _bacc_mod.get_activation_tables

    def _pinned_tables(arch):
        tabs = _orig_gat(arch)
        pin = {
            AF.Square: "silu_and_others",
            AF.Silu: "silu_and_others",
            AF.Abs_reciprocal_sqrt: "abs_reciprocal_sqrt_and_small",
            AF.Ln: "natural_log_exp_and_others",
            AF.Exp: "natural_log_exp_and_others",
        }
        out = {}
        for name, funcs in tabs.items():
            out[name] = {f for f in funcs
                         if f not in pin or pin[f] == name}
        return out

    with tile.TileContext(nc) as tc:
        with (
            tc.tile_pool(name="pconst", bufs=1) as pconst,
            tc.tile_pool(name="px", bufs=4) as px,
            tc.tile_pool(name="py", bufs=3) as py,
            tc.tile_pool(name="pr", bufs=6) as pr,
            tc.tile_pool(name="plt", bufs=2) as plt,
            tc.tile_pool(name="pcv", bufs=2) as pcv,
            tc.tile_pool(name="pdg", bufs=2) as pdg,
            tc.tile_pool(name="pps", bufs=2, space="PSUM") as pps,
            tc.tile_pool(name="ppc", bufs=2, space="PSUM") as ppc,
        ):
            ones = pconst.tile([128, 128], f16)
            nc.sync.dma_start(out=ones[:], in_=ons[:])
            ident = pconst.tile([128, 128], f16)
            nc.sync.dma_start(out=ident[:], in_=idn[:])
            w = pconst.tile([128, GPC, K, DSUB], f32)
            nc.sync.dma_start(out=w[:], in_=wts[:])
            eps_t = pconst.tile([128, 1], f32)
            nc.vector.memset(eps_t[:], EPS)

            xs, ys, rs, mss, yconvs, dgs = {}, {}, {}, {}, {}, {}
            rs_prev = {}

            def stage_load(g, u):
                x = px.tile([128, DSUB, PAD + UT], f16, tag="x")
                xs[(g, u)] = x
                if u == 0:
                    nc.vector.memset(x[:, :, 0:PAD], 0.0)
                    nc.sync.dma_start(
                        out=x[:, :, PAD:PAD + UT],
                        in_=xv[g, :, :, 0:UT])
                else:
                    nc.sync.dma_start(
                        out=x[:],
                        in_=xv[g, :, :, u * UT - PAD:(u + 1) * UT])

            def stage_squares_act(g, u):
                x = xs[(g, u)]
                y = py.tile([128, DSUB, UT], f16, tag="y")
                ys[(g, u)] = y
                nc.scalar.activation(out=y[:, 0:sq_lo, :],
                                     in_=x[:, 0:sq_lo, PAD:PAD + UT],
                                     func=AF.Square)

            def stage_squares_dve(g, u):
                if sq_lo >= DSUB:
                    return
                x, y = xs[(g, u)], ys[(g, u)]
                nc.vector.tensor_tensor(
                    out=y[:, sq_lo:DSUB, :],
                    in0=x[:, sq_lo:DSUB, PAD:PAD + UT],
                    in1=x[:, sq_lo:DSUB, PAD:PAD + UT], op=ALU.mult)

            def stage_mssum(g, u):
                y = ys[(g, u)]
                ms = pps.tile([128, UT], f32, tag="ms")
                for half in range(2):
                    hs = half * 512
                    for s in range(DSUB):
                        nc.tensor.matmul(
                            ms[:, hs:hs + 512], lhsT=ones[:],
                            rhs=y[:, s, hs:hs + 512],
                            start=(s == 0), stop=(s == DSUB - 1))
                mss[(g, u)] = ms

            def stage_rsqrt(g, u):
                ms = mss.pop((g, u))
                r = pr.tile([128, PAD + UT], f16, tag="r")
                if rsqrt_mode == "absrsqrt":
                    nc.scalar.activation(
                        out=r[:, PAD:PAD + UT], in_=ms[:],
                        func=AF.Abs_reciprocal_sqrt,
                        scale=1.0 / D, bias=eps_t[:])
                else:
                    lnt = plt.tile([128, UT], f16, tag="lnt")
                    nc.scalar.activation(out=lnt[:], in_=ms[:], func=AF.Ln,
                                         scale=1.0 / D, bias=eps_t[:])
                    nc.scalar.activation(out=r[:, PAD:PAD + UT], in_=lnt[:],
                                         func=AF.Exp, scale=-0.5)
                rs[(g, u)] = r

            def stage_halo_norm(g, u):
                x, r = xs[(g, u)], rs[(g, u)]
                if u == 0:
                    nc.vector.memset(r[:, 0:PAD], 0.0)
                else:
                    rp = rs_prev[g]
                    nc.vector.tensor_copy(out=r[:, 0:PAD],
                                          in_=rp[:, UT:UT + PAD])
                rb = r[:].rearrange("p (o t) -> p o t", o=1).broadcast_to(
                    [128, DSUB, PAD + UT])
                nc.vector.tensor_tensor(
                    out=x[:], in0=x[:], in1=rb, op=ALU.mult)
                rs_prev[g] = r

            def stage_conv_pe(g, u):
                x = xs[(g, u)]
                dg = dgs[g]
                yps = []
                for si, s in enumerate(pe_s):
                    yp = ppc.tile([128, UT], f32, tag="yp")
                    yps.append(yp)
                    for c in range(UT // 512):
                        cs = c * 512
                        for k in range(K):
                            nc.tensor.matmul(
                                yp[:, cs:cs + 512], lhsT=dg[:, si, k, :],
                                rhs=x[:, s, 2 * k + cs:2 * k + cs + 512],
                                start=(k == 0), stop=(k == K - 1))
                yconvs[(g, u)] = yps

            def stage_conv_dve(g, u):
                x, y = xs[(g, u)], ys[(g, u)]
                for si, s in enumerate(dve_s):
                    nc.vector.tensor_scalar_mul(
                        out=y[:, s, :], in0=x[:, s, 0:UT],
                        scalar1=w[:, g, 0, s:s + 1])
                for k in range(1, K):
                    tmp = pcv.tile([128, ndve, UT], f16, tag="tmp")
                    for si, s in enumerate(dve_s):
                        nc.vector.tensor_scalar_mul(
                            out=tmp[:, si, :], in0=x[:, s, 2 * k:2 * k + UT],
                            scalar1=w[:, g, k, s:s + 1])
                    nc.vector.tensor_tensor(
                        out=y[:, lo:hi, :], in0=y[:, lo:hi, :],
                        in1=tmp[:], op=ALU.add)

            def stage_finish(g, u):
                y = ys[(g, u)]
                yps = yconvs.pop((g, u))
                last = (u == NU - 1)
                done = 0
                nc.scalar.activation(out=y[:, lo:hi, :], in_=y[:, lo:hi, :],
                                     func=AF.Silu)
                for si, s in enumerate(pe_s):
                    nc.scalar.activation(out=y[:, s, :], in_=yps[si][:],
                                         func=AF.Silu)
                    if last and s + 1 in (2, 4, 6):
                        nc.sync.dma_start(
                            out=yv[g, :, done:s + 1, u * UT:(u + 1) * UT],
                            in_=y[:, done:s + 1, :])
                        done = s + 1
                if last:
                    nc.sync.dma_start(
                        out=yv[g, :, done:DSUB, u * UT:(u + 1) * UT],
                        in_=y[:, done:DSUB, :])
                else:
                    nc.sync.dma_start(
                        out=yv[g, :, 0:4, u * UT:(u + 1) * UT],
                        in_=y[:, 0:4, :])
                    nc.sync.dma_start(
                        out=yv[g, :, 4:8, u * UT:(u + 1) * UT],
                        in_=y[:, 4:8, :])

            def stage_diag(g):
                dg = pdg.tile([128, npe, K, 128], f16, tag="dg")
                dgs[g] = dg
                for si, s in enumerate(pe_s):
                    for k in range(K):
                        nc.vector.tensor_scalar_mul(
                            out=dg[:, si, k, :], in0=ident[:],
                            scalar1=w[:, g, k, s:s + 1])

            # 2-wide pipeline: both groups' unit u per step; stats of
            # u+1 interleave with compute of u, per-engine FIFOs ordered so
            # ACT table switches cluster (Square* / AbsRsqrt* / Silu*).
            GS = list(range(GPC))
            stage_diag(0)
            stage_diag(1)
            for gg in GS:
                stage_load(gg, 0)
            for gg in GS:
                stage_squares_act(gg, 0)
            for gg in GS:
                stage_squares_dve(gg, 0)
            for gg in GS:
                stage_mssum(gg, 0)
            for gg in GS:
                stage_rsqrt(gg, 0)
            for u in range(NU):
                nu = u + 1 if u + 1 < NU else None
                if nu is not None:
                    for gg in GS:
                        stage_load(gg, nu)
                    for gg in GS:
                        stage_squares_act(gg, nu)
                for gg in GS:
                    stage_halo_norm(gg, u)
                if nu is not None:
                    for gg in GS:
                        stage_squares_dve(gg, nu)
                for gg in GS:
                    stage_conv_pe(gg, u)
                if nu is not None:
                    for gg in GS:
                        stage_mssum(gg, nu)
                for gg in GS:
                    stage_conv_dve(gg, u)
                for gg in GS:
                    stage_finish(gg, u)
                if nu is not None:
                    for gg in GS:
                        stage_rsqrt(gg, nu)
    _bacc_mod.get_activation_tables = _pinned_tables
    try:
        nc.compile()
    finally:
        _bacc_mod.get_activation_tables = _orig_gat
    return nc


def _host_pack(x, norm_weight, conv_weight):
    xt = np.ascontiguousarray(
        x.astype(np.float16).transpose(0, 2, 3, 1)
    ).reshape(NGROUPS, DSUB, 128, L)
    weff = (conv_weight.reshape(HC, D, K)
            * norm_weight[:, :, None]).astype(np.float32)
    ones = np.ones((128, 128), dtype=np.float16)
    ident = np.eye(128, dtype=np.float16)

    in_maps = []
    for i in range(NCORES):
        gs = [i * GPC + j for j in range(GPC)]
        wc = np.stack([weff[g % HC].reshape(DSUB, 128, K) for g in gs])
        wc = np.ascontiguousarray(wc.transpose(2, 0, 3, 1))
        in_maps.append({
            "xin": np.ascontiguousarray(xt[gs[0]:gs[-1] + 1]),
            "wts": wc,
            "ons": ones,
            "idn": ident,
        })
    return in_maps


def _host_unpack(results):
    ys = np.concatenate([r["yout"] for r in results], axis=0)
    y = ys.reshape(B, HC, D, L).transpose(0, 3, 1, 2)
    return np.ascontiguousarray(y.astype(np.float32))


def _get_prog():
    key = (PE_S, DVE_SQ_S, RSQRT_MODE)
    if key not in _prog_cache:
        _prog_cache[key] = build_program()
    return _prog_cache[key]


def kernel(x, norm_weight, conv_weight, _trace=False, _trace_kwargs=None):
    from concourse.bass_utils import run_bass_kernel_spmd

    x = np.asarray(x, dtype=np.float32)
    norm_weight = np.asarray(norm_weight, dtype=np.float32)
    conv_weight = np.asarray(conv_weight, dtype=np.float32)

    nc = _get_prog()
    in_maps = _host_pack(x, norm_weight, conv_weight)
    res = run_bass_kernel_spmd(
        nc, in_maps, list(range(NCORES)),
        trace=_trace, **(_trace_kwargs or {}))
    out = _host_unpack(res.results)
    if _trace:
        return out, res
    return out
